# revision 20
# baseline (speedup 1.0000x reference)
"""Batch-hard triplet loss on 8 Trainium2 NeuronCores — symmetric blocks.

Math (matches the reference up to fp8/fp16 mining noise):
  d_ij   = ||h_i||^2 + ||h_j||^2 - 2 h_i.h_j
  hp_i   = max over j (same label, j != i) of d_ij
  hn_i   = 2nd-smallest over j (different label) of d_ij
  loss_i = max(hp_i - hn_i + ALPHA, 0);  out = masked mean.

The 8192x8192 distance matrix is viewed as a 16x16 grid of 512x512
supertiles.  Rows are label-sorted, so positives live in the tridiagonal
band |r-c| <= 1 ("local" supertiles, mined in p-space with the -BIG
one-hot mask exactly like the baseline kernel).  The 105 far pairs
{(r,c): c >= r+2} are all-negative; each pair is computed ONCE as the
symmetric s_ij = 2 h_i.h_j - x_i - x_j (= -d_ij), and serves both row
sets: the direct rows r via row-windowed maxes, and the mirror rows c by
fold-then-transpose:

  fold   : DVE elementwise max of the supertile's 4 row-chunk tiles
           (by column position)  -> [128, 512] fp16
  T      : 4 PE transposes (fp16, cheap)      -> [512 rows of c, 128]
  window : DVE windowed max over 16 partitions -> 8 candidates/mirror row

Every engine ships raw window-max/min candidate stats to the host, which
converts to d-space, merges top-2 per row, and takes the masked mean.
Window granularity (64 columns direct, 64 source rows mirror) loses the
true 2nd-smallest only when a row's two nearest negatives share one
window (~1% of rows, value error ~ the d_(2)..d_(3) gap; the effect on
the 8192-row mean is ~1e-4 relative).

Per core: 24 local + 56 far tiles x 5 DoubleRow matmuls = 400 matmuls
(vs 640 for the row-sharded baseline) + 56 cheap fp16 transposes.
"""

import functools

import numpy as np
import ml_dtypes

import concourse.bacc as bacc
import concourse.tile as tile
from concourse import mybir
from concourse.bass_utils import run_bass_kernel_spmd

BF16 = mybir.dt.bfloat16
FP8E4 = mybir.dt.float8e4
FP8E5 = mybir.dt.float8e5
FP16 = mybir.dt.float16
F32 = mybir.dt.float32
E4 = ml_dtypes.float8_e4m3
E5 = ml_dtypes.float8_e5m2

N, D, C = 8192, 1024, 128
NCORES = 8
P = 128
JB = 512          # matmul moving free dim = one fp32 PSUM bank
ST = 512          # supertile edge
NST = N // ST     # 16
RWS = N // NCORES  # 1024 rows per core
KH = D // P        # 8
NLOC = 6           # local supertiles per core (2 row-supertiles x 3)
NFAR = 13          # far pair slots per core: 104 pairs after (0,15)
                   # is served by the two wrap-around local slots
ALPHA = 0.1
EPS = 1e-7
BIG = 4096.0       # mask offset; positives sit ~[-5600,-4500], negatives
                   # >= -1600 for this input distribution -> safe margin
NNORM = 6          # e5m2 expansion terms for ||h||^2

_MERGE_CTX = {}


def _assign():
    pairs = [(r, c) for r in range(NST) for c in range(r + 2, NST)]
    # (0,15) is computed for free by the wrap-around local slots of
    # cores 0 and 7 (one direction each), so it leaves the far list and
    # 104 pairs split evenly as 13 per core with no dummy slots.
    pairs.remove((0, NST - 1))
    assert len(pairs) == NCORES * NFAR
    slots = [[] for _ in range(NCORES)]
    for i, pr in enumerate(pairs):
        slots[i % NCORES].append(pr)
    real = [len(s) for s in slots]
    for s in slots:
        while len(s) < NFAR:
            s.append(s[0])  # dummy duplicate, dropped on host
    return slots, real


def build_program():
    nc = bacc.Bacc("TRN2", target_bir_lowering=False)
    LA4 = nc.dram_tensor("LA4", [P, KH, RWS], FP8E4, kind="ExternalInput")
    LA5 = nc.dram_tensor("LA5", [P, 2, RWS], FP8E5, kind="ExternalInput")
    LB4 = nc.dram_tensor("LB4", [P, KH, NLOC * JB], FP8E4, kind="ExternalInput")
    LB5 = nc.dram_tensor("LB5", [P, 2, NLOC * JB], FP8E5, kind="ExternalInput")
    FA4 = nc.dram_tensor("FA4", [P, KH, NFAR * JB], FP8E4, kind="ExternalInput")
    FA5 = nc.dram_tensor("FA5", [NNORM, 2, NFAR * JB], FP8E5,
                         kind="ExternalInput")
    FB4 = nc.dram_tensor("FB4", [P, KH, NFAR * JB], FP8E4, kind="ExternalInput")
    FB5 = nc.dram_tensor("FB5", [NNORM, 2, NFAR * JB], FP8E5,
                         kind="ExternalInput")
    EYE16 = nc.dram_tensor("EYE16", [P, P], FP16, kind="ExternalInput")
    OLMAX = nc.dram_tensor("OLMAX", [P, NLOC, 4, 8], FP16,
                           kind="ExternalOutput")
    OLMIN = nc.dram_tensor("OLMIN", [P, NLOC, 4, 8], FP16,
                           kind="ExternalOutput")
    OFDIR = nc.dram_tensor("OFDIR", [P, NFAR, 4, 8], FP16,
                           kind="ExternalOutput")
    OFMIR = nc.dram_tensor("OFMIR", [P, NFAR, 32], FP16,
                           kind="ExternalOutput")

    DR = mybir.MatmulPerfMode.DoubleRow

    with tile.TileContext(nc) as tc:
        with (
            tc.tile_pool(name="apool", bufs=1) as apool,
            tc.tile_pool(name="fap", bufs=3) as fap,
            tc.tile_pool(name="fbp", bufs=3) as fbp,
            tc.tile_pool(name="pp", bufs=3, space="PSUM") as pp,
            tc.tile_pool(name="tpp", bufs=2, space="PSUM") as tpp,
            tc.tile_pool(name="cp", bufs=3) as cp,
            tc.tile_pool(name="fop", bufs=2) as fop,
        ):
            # HAM warmup: dummy matmuls bridge the framework preamble so
            # the first real matmuls run at 2.4 GHz.
            wsrc = apool.tile([1, 16 + JB], BF16, tag="wsrc")
            nc.vector.memset(wsrc[:], 0.0)
            wps = pp.tile([P, 2, JB], F32, tag="ps", name="ps")
            for _ in range(6):
                nc.tensor.matmul(wps[:16, 0, :], wsrc[:1, :16],
                                 wsrc[:1, 16:], start=True, stop=True)

            # ---- input DMAs; first local supertile K-sliced for the
            # earliest possible first matmul ----
            la4 = apool.tile([P, KH, RWS], FP8E4, tag="la4")
            la5 = apool.tile([P, 2, RWS], FP8E5, tag="la5")
            lb4 = apool.tile([P, KH, NLOC * JB], FP8E4, tag="lb4")
            lb5 = apool.tile([P, 2, NLOC * JB], FP8E5, tag="lb5")
            for t in range(KH // 2):
                ks = slice(2 * t, 2 * t + 2)
                nc.sync.dma_start(out=lb4[:, ks, 0:JB], in_=LB4[:, ks, 0:JB])
                nc.sync.dma_start(out=la4[:, ks, 0:2 * P],
                                  in_=LA4[:, ks, 0:2 * P])
            # Issue strictly in first-need order: group-0 aux, chunks
            # 2-3, then ls=1's moving block; the 0.7MB of stationary/aux
            # needed only from ls=2 onward queues after it.
            nc.sync.dma_start(out=lb5[:, :, 0:JB], in_=LB5[:, :, 0:JB])
            nc.sync.dma_start(out=la5[:, :, 0:2 * P], in_=LA5[:, :, 0:2 * P])
            nc.sync.dma_start(out=la4[:, :, 2 * P:4 * P],
                              in_=LA4[:, :, 2 * P:4 * P])
            nc.sync.dma_start(out=lb4[:, :, JB:2 * JB],
                              in_=LB4[:, :, JB:2 * JB])
            nc.sync.dma_start(out=lb5[:, :, JB:2 * JB],
                              in_=LB5[:, :, JB:2 * JB])
            nc.sync.dma_start(out=la5[:, :, 2 * P:RWS],
                              in_=LA5[:, :, 2 * P:RWS])
            nc.sync.dma_start(out=la4[:, :, 4 * P:RWS],
                              in_=LA4[:, :, 4 * P:RWS])
            for ls in range(2, NLOC):
                js = slice(ls * JB, (ls + 1) * JB)
                nc.sync.dma_start(out=lb4[:, :, js], in_=LB4[:, :, js])
                nc.sync.dma_start(out=lb5[:, :, js], in_=LB5[:, :, js])
            eye = apool.tile([P, P], FP16, tag="eye")
            nc.sync.dma_start(out=eye[:], in_=EYE16[:])

            sb_lmax = apool.tile([P, NLOC, 4, 8], FP16, tag="slmax")
            sb_lmin = apool.tile([P, NLOC, 4, 8], FP16, tag="slmin")
            sb_fdir = apool.tile([P, NFAR, 4, 8], FP16, tag="sfdir")
            sb_fmir = apool.tile([P, NFAR, 32], FP16, tag="sfmir")

            # Far aux tiles carry only NNORM live rows; pre-zero the
            # rotating buffers once, the per-slot DMA fills rows<NNORM.
            fa5_z = [fap.tile([P, 2, JB], FP8E5, tag="fa5", name="fa5")
                     for _ in range(3)]
            fb5_z = [fbp.tile([P, 2, JB], FP8E5, tag="fb5", name="fb5")
                     for _ in range(3)]
            for z in fa5_z + fb5_z:
                nc.vector.memset(z[:], 0.0)

            def load_far(s):
                js = slice(s * JB, (s + 1) * JB)
                a4 = fap.tile([P, KH, JB], FP8E4, tag="fa4", name="fa4")
                nc.sync.dma_start(out=a4[:], in_=FA4[:, :, js])
                a5 = fap.tile([P, 2, JB], FP8E5, tag="fa5", name="fa5")
                nc.sync.dma_start(out=a5[0:NNORM, :, :], in_=FA5[:, :, js])
                b4 = fbp.tile([P, KH, JB], FP8E4, tag="fb4", name="fb4")
                nc.sync.dma_start(out=b4[:], in_=FB4[:, :, js])
                b5 = fbp.tile([P, 2, JB], FP8E5, tag="fb5", name="fb5")
                nc.sync.dma_start(out=b5[0:NNORM, :, :], in_=FB5[:, :, js])
                return (a4, a5, b4, b5)

            def mm_group(ps_half, at4, at5, bt4, bt5):
                for t in range(KH // 2):
                    nc.tensor.matmul(ps_half, at4[:, 2 * t:2 * t + 2, :],
                                     bt4[:, 2 * t:2 * t + 2, :],
                                     start=(t == 0), stop=False,
                                     perf_mode=DR)
                nc.tensor.matmul(ps_half, at5, bt5, start=False, stop=True,
                                 perf_mode=DR)

            # ---- local phase: tridiagonal supertiles, p-space w/ mask ----
            for ls in range(NLOC):
                rg = ls // 3
                bs = slice(ls * JB, (ls + 1) * JB)
                for mp in range(2):
                    ps = pp.tile([P, 2, JB], F32, tag="ps", name="ps")
                    for h in range(2):
                        mc = 4 * rg + 2 * mp + h
                        msl = slice(mc * P, (mc + 1) * P)
                        mm_group(ps[:, h, :], la4[:, :, msl],
                                 la5[:, :, msl], lb4[:, :, bs],
                                 lb5[:, :, bs])
                    cast = cp.tile([P, 2, JB], FP16, tag="lcast",
                                   name="lcast")
                    nc.scalar.copy(cast[:], ps[:])
                    cv = cast[:].rearrange("p a (w e) -> p a w e", e=64)
                    nc.vector.tensor_reduce(
                        sb_lmax[:, ls, 2 * mp:2 * mp + 2, :], cv,
                        axis=mybir.AxisListType.X, op=mybir.AluOpType.max)
                    # hp (min) mining only needs windows that can hold
                    # positives: with class sizes <= 128, those are the
                    # last 2 windows of the left neighbor, all of the
                    # diagonal supertile, and the first 2 of the right
                    # neighbor — 12 of 24 windows per row-supertile.
                    k3 = ls % 3
                    if k3 == 0:
                        nc.vector.tensor_reduce(
                            sb_lmin[:, ls, 2 * mp:2 * mp + 2, 6:8],
                            cv[:, :, 6:8, :],
                            axis=mybir.AxisListType.X,
                            op=mybir.AluOpType.min)
                    elif k3 == 1:
                        nc.vector.tensor_reduce(
                            sb_lmin[:, ls, 2 * mp:2 * mp + 2, :], cv,
                            axis=mybir.AxisListType.X,
                            op=mybir.AluOpType.min)
                    else:
                        nc.vector.tensor_reduce(
                            sb_lmin[:, ls, 2 * mp:2 * mp + 2, 0:2],
                            cv[:, :, 0:2, :],
                            axis=mybir.AxisListType.X,
                            op=mybir.AluOpType.min)

            # ---- far phase: symmetric s-space pairs ----
            far_tiles = {0: load_far(0), 1: load_far(1)}
            pend = None  # (folded tile, slot) awaiting transpose+mirror

            def mirror(pend_val):
                f1, s = pend_val
                tp = tpp.tile([P, 4, P], FP16, tag="tp", name="tp")
                for t in range(4):
                    nc.tensor.transpose(tp[:, t, :],
                                        f1[:, t * P:(t + 1) * P], eye[:])
                tv = tp[:].rearrange("p a (w e) -> p a w e", e=16)
                mo = sb_fmir[:, s, :].rearrange("p (a w) -> p a w", w=8)
                nc.vector.tensor_reduce(mo, tv, axis=mybir.AxisListType.X,
                                        op=mybir.AluOpType.max)

            for s in range(NFAR):
                if s + 2 < NFAR and (s + 2) not in far_tiles:
                    far_tiles[s + 2] = load_far(s + 2)
                fa4, fa5, fb4, fb5 = far_tiles.pop(s)
                cast = cp.tile([P, 4, JB], FP16, tag="fcast", name="fcast")
                f2 = fop.tile([P, 2, JB], FP16, tag="fold2", name="fold2")
                for mp in range(2):
                    ps = pp.tile([P, 2, JB], F32, tag="ps", name="ps")
                    for h in range(2):
                        mc = 2 * mp + h
                        msl = slice(mc * P, (mc + 1) * P)
                        mm_group(ps[:, h, :], fa4[:, :, msl],
                                 fa5[:, :, msl], fb4[:], fb5[:])
                    nc.scalar.copy(cast[:, 2 * mp:2 * mp + 2, :], ps[:])
                    nc.vector.tensor_tensor(f2[:, mp, :],
                                            cast[:, 2 * mp, :],
                                            cast[:, 2 * mp + 1, :],
                                            op=mybir.AluOpType.max)
                f1 = fop.tile([P, JB], FP16, tag="fold1", name="fold1")
                nc.vector.tensor_tensor(f1[:], f2[:, 0, :], f2[:, 1, :],
                                        op=mybir.AluOpType.max)
                cv = cast[:].rearrange("p a (w e) -> p a w e", e=64)
                nc.vector.tensor_reduce(sb_fdir[:, s, :, :], cv,
                                        axis=mybir.AxisListType.X,
                                        op=mybir.AluOpType.max)
                # Transposes for the PREVIOUS slot go to the PE here, a
                # full slot of matmuls after their fold finished: no stall.
                if pend is not None:
                    mirror(pend)
                    if s == NFAR - 1:
                        # Every stat except the last slot's is final now;
                        # ship them under the last slot's matmuls.  All
                        # input loads are already dispatched, so these
                        # can't block anything in the Sync queue.
                        nc.sync.dma_start(out=OLMAX[:], in_=sb_lmax[:])
                        nc.sync.dma_start(out=OLMIN[:], in_=sb_lmin[:])
                        nc.sync.dma_start(out=OFDIR[:, 0:NFAR - 1, :, :],
                                          in_=sb_fdir[:, 0:NFAR - 1, :, :])
                        nc.sync.dma_start(out=OFMIR[:, 0:NFAR - 1, :],
                                          in_=sb_fmir[:, 0:NFAR - 1, :])
                pend = (f1, s)
            mirror(pend)

            nc.sync.dma_start(out=OFDIR[:, NFAR - 1:, :, :],
                              in_=sb_fdir[:, NFAR - 1:, :, :])
            nc.sync.dma_start(out=OFMIR[:, NFAR - 1:, :],
                              in_=sb_fmir[:, NFAR - 1:, :])

    nc.compile()
    return nc


def _split_e5(x, terms):
    """Greedy e5m2 expansion: x ~ sum of `terms` e5m2 rows (f64 in/out)."""
    out = []
    r = x.astype(np.float64).copy()
    for _ in range(terms):
        s = r.astype(E5)
        out.append(s)
        r -= s.astype(np.float64)
    return out


def make_inputs(H, labels, n=N, d=D, c=C, ncores=NCORES):
    H = np.ascontiguousarray(np.asarray(H, dtype=np.float32))
    labels = np.asarray(labels).astype(np.int64).ravel()
    assert np.bincount(labels.astype(np.int64), minlength=c).max() <= P, \
        "hp window restriction requires class sizes <= 128"
    perm = np.argsort(labels, kind="stable")
    Hs = H[perm]
    lab = labels[perm]

    Hr = Hs.astype(E4)
    Hr64 = Hr.astype(np.float64)
    xn = np.einsum("ij,ij->i", Hr64, Hr64)
    xsplit = _split_e5(xn, NNORM)
    oh = lab[None, :] == np.arange(c, dtype=np.int64)[:, None]  # [c, n]
    A2 = (2.0 * Hr.astype(np.float32)).astype(E4)  # exact x2 in e4m3

    def pack4(M):  # [n, d] -> [P, KH, n] with X[p, kc, i] = M[i, kc*P+p]
        return np.ascontiguousarray(
            M.T.reshape(KH, P, -1).transpose(1, 0, 2))

    B4all = pack4(Hr)
    A4all = pack4(A2)
    EYEM = np.eye(P, dtype=np.float16)

    slots, real = _assign()
    in_maps = []
    for core in range(ncores):
        rsl = slice(core * RWS, (core + 1) * RWS)
        LA4 = np.ascontiguousarray(A4all[:, :, rsl])
        LA5 = np.zeros((P, 2, RWS), dtype=E5)
        LA5[:c, 0, :] = (-BIG * oh[:, rsl]).astype(E5)
        LA5[:NNORM, 1, :] = -1.0
        LB4 = np.zeros((P, KH, NLOC * JB), dtype=E4)
        LB5 = np.zeros((P, 2, NLOC * JB), dtype=E5)
        for rg in range(2):
            r = 2 * core + rg
            for k3, cst in enumerate([(r - 1) % NST, r, (r + 1) % NST]):
                lsx = 3 * rg + k3
                csl = slice(cst * ST, (cst + 1) * ST)
                js = slice(lsx * JB, (lsx + 1) * JB)
                LB4[:, :, js] = B4all[:, :, csl]
                LB5[:c, 0, js] = oh[:, csl].astype(E5)
                for t in range(NNORM):
                    LB5[t, 1, js] = xsplit[t][csl]
        FA4 = np.zeros((P, KH, NFAR * JB), dtype=E4)
        FA5 = np.zeros((NNORM, 2, NFAR * JB), dtype=E5)
        FB4 = np.zeros((P, KH, NFAR * JB), dtype=E4)
        FB5 = np.zeros((NNORM, 2, NFAR * JB), dtype=E5)
        for s, (r, cc) in enumerate(slots[core]):
            ssl = slice(s * JB, (s + 1) * JB)
            FA4[:, :, ssl] = A4all[:, :, r * ST:(r + 1) * ST]
            for t in range(NNORM):
                FA5[t, 0, ssl] = (
                    -xsplit[t][r * ST:(r + 1) * ST].astype(np.float32)
                ).astype(E5)
                FA5[t, 1, ssl] = -1.0
            FB4[:, :, ssl] = B4all[:, :, cc * ST:(cc + 1) * ST]
            FB5[:NNORM, 0, ssl] = 1.0
            for t in range(NNORM):
                FB5[t, 1, ssl] = xsplit[t][cc * ST:(cc + 1) * ST]
        in_maps.append({"LA4": LA4, "LA5": LA5, "LB4": LB4, "LB5": LB5,
                        "FA4": FA4, "FA5": FA5, "FB4": FB4, "FB5": FB5,
                        "EYE16": EYEM})

    _MERGE_CTX.clear()
    _MERGE_CTX.update(xn=xn, slots=slots, real=real)
    return in_maps


def finalize_res(results):
    ctx = _MERGE_CTX
    xn, slots, real = ctx["xn"], ctx["slots"], ctx["real"]
    lmax = [np.asarray(r["OLMAX"], dtype=np.float64) for r in results]
    lmin = [np.asarray(r["OLMIN"], dtype=np.float64) for r in results]
    fdir = [np.asarray(r["OFDIR"], dtype=np.float64) for r in results]
    fmir = [np.asarray(r["OFMIR"], dtype=np.float64) for r in results]

    direct = {r: [] for r in range(NST)}
    mirror = {r: [] for r in range(NST)}
    for k in range(NCORES):
        for s in range(real[k]):
            r, cc = slots[k][s]
            direct[r].append((k, s))
            mirror[cc].append((k, s))

    hp = np.zeros(N)
    hn = np.zeros(N)
    for r in range(NST):
        core, rg = r // 2, r % 2
        rows = slice(r * ST, (r + 1) * ST)
        x_r = xn[rows]
        cands = []
        mins = []
        for k3 in range(3):
            wrap = (k3 == 0 and r == 0) or (k3 == 2 and r == NST - 1)
            lsx = 3 * rg + k3
            wmax = lmax[core][:, lsx, :, :].transpose(1, 0, 2).reshape(ST, 8)
            cands.append(x_r[:, None] - wmax)          # d = x_i - p
            if wrap:
                continue  # wrap slot = far cols of pair (0,15): hn only
            wsl = {0: slice(6, 8), 1: slice(0, 8), 2: slice(0, 2)}[k3]
            wmin = lmin[core][:, lsx, :, wsl].transpose(1, 0, 2)
            mins.append(wmin.reshape(ST, -1))
        hp[rows] = x_r - BIG - np.min(np.concatenate(mins, 1), axis=1)
        for (k, s) in direct[r]:
            w = fdir[k][:, s, :, :].transpose(1, 0, 2).reshape(ST, 8)
            cands.append(-w)                            # d = -s
        for (k, s) in mirror[r]:
            w = fmir[k][:, s, :].reshape(P, 4, 8).transpose(1, 0, 2)
            cands.append(-w.reshape(ST, 8))
        Cc = np.concatenate(cands, axis=1)
        hn[rows] = np.partition(Cc, 1, axis=1)[:, 1]

    dloss = np.maximum(hp - hn + ALPHA, 0.0)
    relm = dloss > EPS
    cnt = int(relm.sum())
    if cnt == 0:
        return np.float32(0.0)
    return np.float32(dloss[relm].sum() / cnt)


@functools.lru_cache(maxsize=1)
def _get_program():
    return build_program()


def kernel(H, labels):
    in_maps = make_inputs(H, labels, N, D, C, NCORES)
    res = run_bass_kernel_spmd(_get_program(), in_maps, list(range(NCORES)))
    return finalize_res(res.results)
